# revision 7
# baseline (speedup 1.0000x reference)
"""Bayesian NN Monte-Carlo sampling kernel for 8 TRN2 NeuronCores.

Shards the n_samples axis (S=100 -> 13 per core, 4 padded) across 8 cores.
Per core, for each sample s:
  W_s = eps_s * exp(0.5*logvar) + mean   (DVE does the eps*std mult; the
                                          "+ mean" is folded into the matmul
                                          as a second accumulating matmul)
  act = relu(x @ W0_s + b0_s); act = relu(act @ W1_s + b1_s); out = act @ Wl_s + bl_s
Activations keep batch (64) on PSUM partitions; weights are the moving matmul
operand (float32r streams 1 row/cycle at N=512). Biases are sampled once for
all 13 samples in a [13, D] tile and injected per sample with an indicator
matmul (lhsT = e_s \otimes ones_64, K=13).
"""

import os
import sys

import numpy as np

if "/opt/trn_rl_repo" not in sys.path:
    sys.path.insert(0, "/opt/trn_rl_repo")

import concourse.bass as bass
from concourse import bacc, mybir, tile
from concourse.bass_utils import run_bass_kernel_spmd

S, B = 100, 64
D0, D1, D2, DO = 784, 512, 512, 10
NCORES = 8
SP = 13          # samples per core; 8*13 = 104, last 4 are padding
KT0, K0 = 7, 112  # layer-0 contraction tiling: 7 tiles x 112 = 784
KT1, K1 = 4, 128  # layer-1/2 contraction tiling: 4 tiles x 128 = 512

F32 = mybir.dt.float32
F32R = mybir.dt.float32r
BF16 = mybir.dt.bfloat16

# "f32r": fp32 storage/DMA, float32r matmuls (fast PE, near-fp32 accuracy)
# "bf16": bf16 storage/DMA (half memory traffic), bf16 matmuls
DTYPE_MODE = os.environ.get("BNN_DTYPE", "f32r")

_CACHE = {}


def _build(mode):
    # In f32r mode every matmul-feeding tensor is declared float32r (same
    # 4-byte storage as fp32; walrus requires producers of fp32r-matmul
    # operands to be fp32r-typed so their outputs are mantissa-rounded).
    io_dt = BF16 if mode == "bf16" else F32R
    mm_dt = BF16 if mode == "bf16" else F32R
    ts = bass.ts

    nc = bacc.Bacc("TRN2", target_bir_lowering=False, debug=False,
                   num_devices=NCORES)

    def inp(name, shape):
        return nc.dram_tensor(name, shape, io_dt, kind="ExternalInput").ap()

    xT = inp("xT", [D0, B])
    wm0 = inp("wm0", [D0, D1])
    wv0 = inp("wv0", [D0, D1])
    wm1 = inp("wm1", [D1, D2])
    wv1 = inp("wv1", [D1, D2])
    wmlT = inp("wmlT", [K1, KT1 * DO])      # [128, 40] chunk-major
    wvlT = inp("wvlT", [K1, KT1 * DO])
    welT = inp("welT", [K1, SP * KT1 * DO])  # [128, 13*40]
    we0 = inp("we0", [SP, D0, D1])
    we1 = inp("we1", [SP, D1, D2])
    bv0 = inp("bv0", [1, D1])
    bm0 = inp("bm0", [1, D1])
    bv1 = inp("bv1", [1, D2])
    bm1 = inp("bm1", [1, D2])
    bvl = inp("bvl", [1, DO])
    bml = inp("bml", [1, DO])
    be0 = inp("be0", [SP, D1])
    be1 = inp("be1", [SP, D2])
    bel = inp("bel", [SP, DO])
    id64 = inp("id64", [B, B])
    ind = inp("ind", [SP, SP * B])
    ones13 = inp("ones13", [1, SP])
    out = nc.dram_tensor("out", [B, SP * DO], F32, kind="ExternalOutput").ap()

    AF = mybir.ActivationFunctionType

    with tile.TileContext(nc) as tc:
        with tc.tile_pool(name="const", bufs=1) as const, \
             tc.tile_pool(name="w0e", bufs=3) as w0e, \
             tc.tile_pool(name="w0s", bufs=2) as w0s, \
             tc.tile_pool(name="w1e", bufs=3) as w1e, \
             tc.tile_pool(name="w1s", bufs=2) as w1s, \
             tc.tile_pool(name="wls", bufs=2) as wls, \
             tc.tile_pool(name="acts", bufs=2) as acts, \
             tc.tile_pool(name="bias", bufs=1) as bias, \
             tc.tile_pool(name="ps_mm", bufs=3, space="PSUM") as ps_mm, \
             tc.tile_pool(name="ps_tr", bufs=2, space="PSUM") as ps_tr, \
             tc.tile_pool(name="ps_o", bufs=2, space="PSUM") as ps_o:

            # ---------------- one-time setup ----------------
            t_xT = const.tile([K0, KT0 * B], io_dt)
            nc.sync.dma_start(t_xT[:].rearrange("p (t b) -> p t b", t=KT0),
                              xT.rearrange("(t p) b -> p t b", p=K0))

            # std0 = exp(0.5*wv0), resident [112, 7*512]
            tmp0 = w0e.tile([K0, KT0 * D1], io_dt, tag="t_we0")
            nc.sync.dma_start(tmp0[:].rearrange("p (t n) -> p t n", t=KT0),
                              wv0.rearrange("(t p) n -> p t n", p=K0))
            t_std0 = const.tile([K0, KT0 * D1], io_dt)
            nc.scalar.activation(t_std0[:], tmp0[:], AF.Exp, scale=0.5)
            t_wm0 = const.tile([K0, KT0 * D1], io_dt)
            nc.sync.dma_start(t_wm0[:].rearrange("p (t n) -> p t n", t=KT0),
                              wm0.rearrange("(t p) n -> p t n", p=K0))

            tmp1 = w1e.tile([K1, KT1 * D2], io_dt, tag="t_we1")
            nc.sync.dma_start(tmp1[:].rearrange("p (t n) -> p t n", t=KT1),
                              wv1.rearrange("(t p) n -> p t n", p=K1))
            t_std1 = const.tile([K1, KT1 * D2], io_dt)
            nc.scalar.activation(t_std1[:], tmp1[:], AF.Exp, scale=0.5)
            t_wm1 = const.tile([K1, KT1 * D2], io_dt)
            nc.sync.dma_start(t_wm1[:].rearrange("p (t n) -> p t n", t=KT1),
                              wm1.rearrange("(t p) n -> p t n", p=K1))

            tmpl = wls.tile([K1, KT1 * DO], io_dt, tag="t_wls")
            nc.sync.dma_start(tmpl[:], wvlT[:, :])
            t_stdl = const.tile([K1, KT1 * DO], io_dt)
            nc.scalar.activation(t_stdl[:], tmpl[:], AF.Exp, scale=0.5)
            t_wml = const.tile([K1, KT1 * DO], io_dt)
            nc.sync.dma_start(t_wml[:], wmlT[:, :])
            t_wel = const.tile([K1, SP * KT1 * DO], io_dt)
            nc.sync.dma_start(t_wel[:], welT[:, :])

            # biases: sample all SP at once in [SP, D] tiles; the [1,D] row
            # is broadcast to SP partitions with a K=1 ones-matmul.
            t_ones13 = const.tile([1, SP], io_dt)
            nc.sync.dma_start(t_ones13[:], ones13[:, :])

            def bcast(row, D, tag):
                pb = ps_mm.tile([SP, D], F32, tag="mm")
                nc.tensor.matmul(pb[:], t_ones13[:], row[:],
                                 start=True, stop=True)
                sbuf = bias.tile([SP, D], io_dt, tag=tag)
                nc.scalar.copy(sbuf[:], pb[:])
                return sbuf

            def make_bias(bv, bm, be, D, layer):
                r = bias.tile([1, D], io_dt, tag="brow")
                nc.sync.dma_start(r[:], bv[:, :])
                sb = bias.tile([1, D], io_dt, tag="brow2")
                nc.scalar.activation(sb[:], r[:], AF.Exp, scale=0.5)
                sbb = bcast(sb, D, "bb1")
                mr = bias.tile([1, D], io_dt, tag="brow3")
                nc.sync.dma_start(mr[:], bm[:, :])
                mb = bcast(mr, D, "bb2")
                eb = bias.tile([SP, D], io_dt, tag="bb3")
                nc.sync.dma_start(eb[:], be[:, :])
                ba = bias.tile([SP, D], io_dt, tag="bb4")
                nc.vector.tensor_mul(ba[:], eb[:], sbb[:])
                ball = bias.tile([SP, D], io_dt, tag=f"ball_{layer}")
                nc.vector.tensor_add(ball[:], ba[:], mb[:])
                return ball

            t_b0 = make_bias(bv0, bm0, be0, D1, 0)
            t_b1 = make_bias(bv1, bm1, be1, D2, 1)
            t_bl = make_bias(bvl, bml, bel, DO, 2)

            # indicator: ind[k, s*64+b] = 1 if k == s else 0 (host-built)
            t_ind = const.tile([SP, SP * B], io_dt)
            nc.sync.dma_start(t_ind[:], ind[:, :])

            t_id = const.tile([B, B], io_dt)
            nc.sync.dma_start(t_id[:], id64[:, :])

            t_zb = const.tile([B, 1], F32)
            nc.vector.memset(t_zb[:], 0.0)

            t_out = const.tile([B, SP * DO], F32)

            def mm(psum, lhsT, rhs, start, stop):
                nc.tensor.matmul(psum, lhsT.bitcast(mm_dt), rhs.bitcast(mm_dt),
                                 start=start, stop=stop)

            # ---------------- per-sample pipeline ----------------
            for s in range(SP):
                # ---- layer 0 ----
                t_we0 = w0e.tile([K0, KT0 * D1], io_dt, tag="t_we0")
                nc.sync.dma_start(
                    t_we0[:].rearrange("p (t n) -> p t n", t=KT0),
                    we0[s].rearrange("(t p) n -> p t n", p=K0))
                t_w0 = w0s.tile([K0, KT0 * D1], io_dt)
                nc.vector.tensor_mul(t_w0[:], t_we0[:], t_std0[:])

                p0 = ps_mm.tile([B, D1], F32, tag="mm")
                for t in range(KT0):
                    mm(p0[:], t_xT[:, ts(t, B)], t_w0[:, ts(t, D1)],
                       start=(t == 0), stop=False)
                for t in range(KT0):
                    mm(p0[:], t_xT[:, ts(t, B)], t_wm0[:, ts(t, D1)],
                       start=False, stop=False)
                mm(p0[:], t_ind[:, ts(s, B)], t_b0[:], start=False, stop=True)

                a1 = acts.tile([B, D1], io_dt, tag="a1")
                nc.scalar.activation(a1[:], p0[:], AF.Relu, bias=t_zb[:])

                ptr1 = ps_tr.tile([K1, KT1 * B], io_dt, tag="tr")
                for c in range(KT1):
                    nc.tensor.transpose(ptr1[:, ts(c, B)], a1[:, ts(c, K1)],
                                        t_id[:])
                a1T = acts.tile([K1, KT1 * B], io_dt, tag="a1T")
                nc.scalar.copy(a1T[:], ptr1[:])

                # ---- layer 1 ----
                t_we1 = w1e.tile([K1, KT1 * D2], io_dt, tag="t_we1")
                nc.sync.dma_start(
                    t_we1[:].rearrange("p (t n) -> p t n", t=KT1),
                    we1[s].rearrange("(t p) n -> p t n", p=K1))
                t_w1 = w1s.tile([K1, KT1 * D2], io_dt)
                nc.vector.tensor_mul(t_w1[:], t_we1[:], t_std1[:])

                p1 = ps_mm.tile([B, D2], F32, tag="mm")
                for t in range(KT1):
                    mm(p1[:], a1T[:, ts(t, B)], t_w1[:, ts(t, D2)],
                       start=(t == 0), stop=False)
                for t in range(KT1):
                    mm(p1[:], a1T[:, ts(t, B)], t_wm1[:, ts(t, D2)],
                       start=False, stop=False)
                mm(p1[:], t_ind[:, ts(s, B)], t_b1[:], start=False, stop=True)

                a2 = acts.tile([B, D2], io_dt, tag="a2")
                nc.scalar.activation(a2[:], p1[:], AF.Relu, bias=t_zb[:])

                ptr2 = ps_tr.tile([K1, KT1 * B], io_dt, tag="tr")
                for c in range(KT1):
                    nc.tensor.transpose(ptr2[:, ts(c, B)], a2[:, ts(c, K1)],
                                        t_id[:])
                a2T = acts.tile([K1, KT1 * B], io_dt, tag="a2T")
                nc.scalar.copy(a2T[:], ptr2[:])

                # ---- output layer ----
                t_wl = wls.tile([K1, KT1 * DO], io_dt, tag="t_wls")
                nc.vector.tensor_mul(t_wl[:], t_wel[:, ts(s, KT1 * DO)],
                                     t_stdl[:])
                po = ps_o.tile([B, DO], F32, tag="out")
                for t in range(KT1):
                    mm(po[:], a2T[:, ts(t, B)], t_wl[:, ts(t, DO)],
                       start=(t == 0), stop=False)
                for t in range(KT1):
                    mm(po[:], a2T[:, ts(t, B)], t_wml[:, ts(t, DO)],
                       start=False, stop=False)
                mm(po[:], t_ind[:, ts(s, B)], t_bl[:], start=False, stop=True)

                nc.scalar.copy(t_out[:, ts(s, DO)], po[:])

            nc.sync.dma_start(out[:, :], t_out[:])

    nc.compile()
    return nc


def _get_nc(mode):
    if mode not in _CACHE:
        _CACHE[mode] = _build(mode)
    return _CACHE[mode]


def _prep_in_maps(inputs, mode):
    np_dt = np.float32
    if mode == "bf16":
        import ml_dtypes
        np_dt = ml_dtypes.bfloat16

    def cvt(a):
        return np.ascontiguousarray(a).astype(np_dt, copy=False)

    x = np.asarray(inputs["inputs"], np.float32)
    we0 = np.asarray(inputs["we0"], np.float32)
    we1 = np.asarray(inputs["we1"], np.float32)
    wel = np.asarray(inputs["wel"], np.float32)
    be0 = np.asarray(inputs["be0"], np.float32).reshape(S, D1)
    be1 = np.asarray(inputs["be1"], np.float32).reshape(S, D2)
    bel = np.asarray(inputs["bel"], np.float32).reshape(S, DO)

    shared = {
        "xT": cvt(x.T),
        "wm0": cvt(inputs["wm0"]),
        "wv0": cvt(inputs["wv0"]),
        "wm1": cvt(inputs["wm1"]),
        "wv1": cvt(inputs["wv1"]),
        "wmlT": cvt(np.asarray(inputs["wml"], np.float32)
                    .reshape(KT1, K1, DO).transpose(1, 0, 2).reshape(K1, KT1 * DO)),
        "wvlT": cvt(np.asarray(inputs["wvl"], np.float32)
                    .reshape(KT1, K1, DO).transpose(1, 0, 2).reshape(K1, KT1 * DO)),
        "bv0": cvt(np.asarray(inputs["bv0"], np.float32).reshape(1, D1)),
        "bm0": cvt(np.asarray(inputs["bm0"], np.float32).reshape(1, D1)),
        "bv1": cvt(np.asarray(inputs["bv1"], np.float32).reshape(1, D2)),
        "bm1": cvt(np.asarray(inputs["bm1"], np.float32).reshape(1, D2)),
        "bvl": cvt(np.asarray(inputs["bvl"], np.float32).reshape(1, DO)),
        "bml": cvt(np.asarray(inputs["bml"], np.float32).reshape(1, DO)),
        "id64": cvt(np.eye(B, dtype=np.float32)),
        "ind": cvt(np.repeat(np.eye(SP, dtype=np.float32), B, axis=1)),
        "ones13": cvt(np.ones((1, SP), np.float32)),
    }

    def shard(a, k):
        lo = k * SP
        hi = lo + SP
        if hi <= S:
            return a[lo:hi]
        return np.concatenate([a[lo:S], a[: hi - S]], axis=0)

    in_maps = []
    for k in range(NCORES):
        welk = shard(wel, k)  # [SP, 512, 10]
        in_maps.append(dict(
            shared,
            we0=cvt(shard(we0, k)),
            we1=cvt(shard(we1, k)),
            welT=cvt(welk.reshape(SP, KT1, K1, DO).transpose(2, 0, 1, 3)
                     .reshape(K1, SP * KT1 * DO)),
            be0=cvt(shard(be0, k)),
            be1=cvt(shard(be1, k)),
            bel=cvt(shard(bel, k)),
        ))
    return in_maps


def _run(inputs, mode=DTYPE_MODE, trace=False):
    nc = _get_nc(mode)
    in_maps = _prep_in_maps(inputs, mode)
    res = run_bass_kernel_spmd(nc, in_maps, core_ids=list(range(NCORES)),
                               trace=trace)
    outs = []
    for k in range(NCORES):
        o = np.asarray(res.results[k]["out"], np.float32)  # [64, 130]
        outs.append(o.reshape(B, SP, DO).transpose(1, 0, 2))
    full = np.concatenate(outs, axis=0)[:S]  # [100, 64, 10]
    return full, res


def kernel(**inputs):
    out, _ = _run(inputs)
    return out
